# revision 40
# baseline (speedup 1.0000x reference)
"""NearAggregator Trainium2 Bass kernel (hybrid-layout, diag-free pooling).

Math (per batch item b):
    Kcat   = concat([near_emb, delta_xy, delta_cs], -1)          # [N, 132]
    scores = (Kcat @ W_key + b_key) . B_query[b] / sqrt(64)      # [N]
    out[b] = softmax(scores) @ near_emb[b]                       # [128]

Reformulations:
  * Fold W_key into the query side: qp[b,:] = 0.125 * (W_key^T @ B_query[b])
    (132-dim), so scores[b,n] = Kcat[b,n,:] . qp[b,:].
  * b_key shifts scores by a per-b constant -> softmax-invariant -> dropped.
  * softmax without max-subtraction: scores ~ N(0, 0.58), exp safe in fp32.
  * Normalization (1/sumexp) folded into the final PSUM->SBUF output copy.

Hybrid layout: SBUF tiles hold [(16 items x 8 n_hi) partitions,
(16 n_lo "slabs" x 132 features)].  Host packs near|dxy|dcs into one
[B, N, 132] tensor so each partition's data is one contiguous 8448B
DMA descriptor.  Neighbor n = n_hi*16 + n_lo.

Per 16-item chunk:
  scores : 16 DVE scalar_tensor_tensor ops (in1 = qp replicated across
           n_hi via a broadcast stationary in the qp matmul), accum_out
           gives sc[(i,n_hi), n_lo].
  softmax: Act exp with accum -> partials over n_lo; PE matmul with
           constant M16 (item-selection 0/1 matrix) sums over n_hi;
           DVE reciprocal.
  pooling: Pool-engine tensor_tensor builds stationary weights
           stat[(i,n_hi), (slab, j)] = e * M16 (bf16); Act casts the
           near part of the mega-tile to bf16; 16 PE matmuls
           (stationary [128,16], moving [128,128]) accumulate
           pooled[16 items, 128] in PSUM -- no diag matrices at all.
  output : Act copy-with-scale (x 1/sumexp) PSUM->SBUF, DMA out.

Data parallel over 8 NeuronCores: batch 8192 -> 1024 per core.
"""

import os

import numpy as np

B = 8192
N = 128
D = 128
DQ = 64
F = D + 4
CORES = 8
PB = B // CORES            # 1024 items per core
CH = 16                    # items per chunk (partition dim = CH * NH)
NH = 128 // CH             # 8 n_hi replicas per item
NSLAB = N // CH            # 16 n_lo slabs per chunk... (= N // CH only if CH*NH=128)
NCHUNK = PB // CH          # 64 chunks per core
GI = 128 // CH             # chunks per item-group of 128 (bq transpose granularity)

# out of every 8 chunks, how many bf16 casts run on the Pool engine (vs Act)
POOL_CASTS = int(os.environ.get("NK_POOL_CASTS", "6"))

_NC = None


def _build():
    import concourse.tile as tile
    from concourse import bacc, mybir

    f32 = mybir.dt.float32
    bf16 = mybir.dt.bfloat16
    mult = mybir.AluOpType.mult
    bypass = mybir.AluOpType.bypass

    assert CH * NH == 128
    NLO = N // NH            # 16 neighbors (slabs) per partition-row

    nc = bacc.Bacc(
        "TRN2",
        target_bir_lowering=False,
        debug=False,
        enable_asserts=True,
        num_devices=CORES,
    )
    nearcat = nc.dram_tensor("nearcat", [PB, N, F], f32, kind="ExternalInput").ap()
    bq = nc.dram_tensor("bq", [PB, DQ], f32, kind="ExternalInput").ap()
    wk = nc.dram_tensor("wk", [F, DQ], f32, kind="ExternalInput").ap()
    out = nc.dram_tensor("out", [PB, D], f32, kind="ExternalOutput").ap()
    ident_dram = nc.inline_tensor(np.eye(128, dtype=np.float32), name="ident").ap()
    # M16[p, j] = 1 if item(p) == j else 0, p = (i, n_hi) i-major
    m16_np = np.kron(np.eye(CH, dtype=np.float32), np.ones((NH, 1), np.float32))
    m16_dram = nc.inline_tensor(m16_np, name="m16").ap()
    # SELALL[j, cc*128 + p] = 1 iff j == cc*CH + item(p): per-chunk qp
    # replication matmul stationary (one free dim each [128, 128] slice).
    sel_np = np.zeros((128, GI * 128), dtype=np.float32)
    for cc in range(GI):
        for p in range(128):
            sel_np[cc * CH + p // NH, cc * 128 + p] = 1.0
    sel_dram = nc.inline_tensor(sel_np, name="selall").ap()

    stage = int(os.environ.get("NK_STAGE", "9"))
    nchunk = int(os.environ.get("NK_CHUNKS", str(NCHUNK)))

    with tile.TileContext(nc) as tc:
        from contextlib import ExitStack

        ctx = ExitStack()
        with ctx:
            consts = ctx.enter_context(tc.tile_pool(name="consts", bufs=1))
            tmeg = ctx.enter_context(tc.tile_pool(name="tmeg", bufs=4))
            tbp = ctx.enter_context(tc.tile_pool(name="tbp", bufs=4))
            qpp = ctx.enter_context(tc.tile_pool(name="qpp", bufs=3))
            prp = ctx.enter_context(tc.tile_pool(name="prp", bufs=4))
            scp = ctx.enter_context(tc.tile_pool(name="scp", bufs=3))
            statp = ctx.enter_context(tc.tile_pool(name="statp", bufs=3))
            outp = ctx.enter_context(tc.tile_pool(name="outp", bufs=3))
            bqp = ctx.enter_context(tc.tile_pool(name="bqp", bufs=2))
            psq = ctx.enter_context(tc.tile_pool(name="psq", bufs=2, space="PSUM"))
            psb = ctx.enter_context(tc.tile_pool(name="psb", bufs=1, space="PSUM"))
            psp = ctx.enter_context(tc.tile_pool(name="psp", bufs=2, space="PSUM"))
            pss = ctx.enter_context(tc.tile_pool(name="pss", bufs=1, space="PSUM"))

            # ---- one-time setup ----
            identity = consts.tile([128, 128], f32)
            nc.scalar.dma_start(identity[:], ident_dram[:])
            m16 = consts.tile([128, CH], f32)
            nc.scalar.dma_start(m16[:], m16_dram[:])
            # [M16 | M16]: 32-wide so each sums matmul fills a full 32-row band
            m32 = consts.tile([128, 2 * CH], f32)
            nc.scalar.dma_start(m32[:, 0:CH], m16_dram[:])
            nc.scalar.dma_start(m32[:, CH : 2 * CH], m16_dram[:])
            selall = consts.tile([128, GI * 128], f32)
            nc.scalar.dma_start(selall[:], sel_dram[:])

            # wT = 0.125 * W_key^T  as [64, 132]
            w1 = consts.tile([128, DQ], f32)
            nc.scalar.dma_start(w1[:], wk[0:128, :])
            w2 = consts.tile([4, DQ], f32)
            nc.scalar.dma_start(w2[:], wk[128:132, :])
            wT = consts.tile([DQ, F], f32)
            stp = pss.tile([DQ, 128], f32, tag="setup_ps")
            nc.tensor.transpose(stp[:], w1[:], identity[:])
            nc.scalar.mul(wT[:, 0:128], stp[:], 0.125)
            stp2 = pss.tile([DQ, 4], f32, tag="setup_ps")
            nc.tensor.transpose(stp2[:], w2[:], identity[0:4, 0:4])
            nc.scalar.mul(wT[:, 128:132], stp2[:], 0.125)

            # ---- all bq loaded once: bqall[p, g, :] = bq[g*128 + p] ----
            ngroups = nchunk // GI
            bqall = consts.tile([128, max(ngroups, 1), DQ], f32)
            nc.scalar.dma_start(
                bqall[:, 0:ngroups, :],
                bq[0 : ngroups * 128, :].rearrange("(g p) q -> p g q", g=ngroups),
            )

            SUP = 4                  # chunks per super-chunk (psum partition bands)
            pending_tail = [None]    # deferred super-chunk finalization

            def flush_tail():
                if pending_tail[0] is not None:
                    pending_tail[0]()
                    pending_tail[0] = None

            for g in range(ngroups):
                # ---- per-group: qpg[item, 132] = 0.125 * bq @ W^T ----
                bqT_ps = psb.tile([DQ, 128], f32, tag="gps")
                nc.tensor.transpose(bqT_ps[:], bqall[:, g, :], identity[:])
                bqT = bqp.tile([DQ, 128], f32, tag="bqts")
                nc.scalar.copy(bqT[:], bqT_ps[:])
                qpg_ps = psb.tile([128, F], f32, tag="gps")
                nc.tensor.matmul(qpg_ps[:], bqT[:], wT[:], start=True, stop=True)
                qpg = bqp.tile([128, F], f32, tag="qpg")
                nc.scalar.copy(qpg[:], qpg_ps[:])

                pooled_sup = None
                sums_sup = None
                outg = None
                for gi in range(GI):
                    c = g * GI + gi
                    b0 = c * CH
                    q = gi % SUP     # band within super-chunk
                    if q == 0:
                        pooled_sup = psp.tile([128, D], f32, tag="pool")
                        sums_sup = pss.tile([128, 1], f32, tag="sums")
                        outg = outp.tile([128, D], f32, tag="out")

                    # ---- T mega-tile: one DMA loads 2 chunks
                    # free dim cp selects the chunk; partitions stay (i, n_hi)
                    # with i relative to each chunk's 16 items.
                    if q % 2 == 0:
                        tmsup = tmeg.tile(
                            [128, 2, NLO, F], f32, name=f"tm{c}", tag="tm"
                        )
                        src = nearcat[b0 : b0 + 2 * CH].rearrange(
                            "(cp i) (nh s) f -> (i nh) cp s f", cp=2, nh=NH
                        )
                        nc.sync.dma_start(tmsup[:], src)
                    tm = tmsup[:, q % 2, :, :]

                    if stage <= 1:
                        ot = outp.tile([CH, D], f32, tag="dbg")
                        nc.vector.tensor_copy(ot[:], tm[0:CH, 0, 0:128])
                        nc.sync.dma_start(out[b0 : b0 + CH, :], ot[:])
                        continue

                    # ---- qpR[(i,nh), 132] = qpg[chunk item] via selection matmul ----
                    qp_ps = psq.tile([128, F], f32, tag="qpps")
                    nc.tensor.matmul(
                        qp_ps[:],
                        selall[:, gi * 128 : (gi + 1) * 128],
                        qpg[:],
                        start=True,
                        stop=True,
                    )
                    qp = qpp.tile([128, F], f32, tag="qp")
                    nc.scalar.copy(qp[:], qp_ps[:])

                    if stage <= 2:
                        ot = outp.tile([CH, D], f32, tag="dbg")
                        nc.vector.tensor_copy(ot[:], qp[0:CH, 0:128])
                        nc.sync.dma_start(out[b0 : b0 + CH, :], ot[:])
                        continue

                    # ---- scores sc[(i,nh), s] via fused STT + accum ----
                    sc = scp.tile([128, NLO], f32, tag="sc")
                    for s in range(NLO):
                        pr = prp.tile([128, F], bf16, name=f"pr{c}_{s}", tag="pr")
                        nc.vector.scalar_tensor_tensor(
                            out=pr[:],
                            in0=tm[:, s, :],
                            scalar=1.0,
                            in1=qp[:],
                            op0=bypass,
                            op1=mult,
                            accum_out=sc[:, s : s + 1],
                        )

                    if stage <= 3:
                        ot = outp.tile([CH, D], f32, tag="dbg")
                        nc.vector.tensor_copy(
                            ot[:], sc[0:CH, :].to_broadcast([CH, D])
                        )
                        nc.sync.dma_start(out[b0 : b0 + CH, :], ot[:])
                        continue

                    # ---- bf16 cast of near part ----
                    tb = tbp.tile([128, NLO, D], bf16, name=f"tb{c}", tag="tb")
                    if (c % 8) < POOL_CASTS:
                        nc.gpsimd.tensor_copy(tb[:], tm[:, :, 0:128])
                    else:
                        nc.scalar.copy(tb[:], tm[:, :, 0:128])

                    # ---- softmax pieces ----
                    e_t = scp.tile([128, NLO], f32, tag="et")
                    partials = scp.tile([128, 1], f32, tag="pt")
                    nc.scalar.activation(
                        e_t[:],
                        sc[:],
                        func=mybir.ActivationFunctionType.Exp,
                        accum_out=partials[:],
                    )
                    # sums for band q land at psum partitions 32q..32q+16
                    nc.tensor.matmul(
                        sums_sup[32 * q : 32 * q + 32, :],
                        m32[:],
                        partials[:],
                        start=True,
                        stop=True,
                        tile_position=(0, 32 * q),
                    )

                    # previous super-chunk's finalization goes here, AFTER this
                    # chunk's score STTs, so the DVE reciprocal doesn't
                    # head-of-line block the score stream.
                    flush_tail()

                    if stage <= 4:
                        ot = outp.tile([CH, D], f32, tag="dbg")
                        nc.vector.tensor_copy(
                            ot[:], e_t[0:CH, :].to_broadcast([CH, D])
                        )
                        nc.sync.dma_start(out[b0 : b0 + CH, :], ot[:])
                        continue

                    # ---- stationary weights stat[(i,nh), (s, j)] = e * M32 ----
                    # 32-wide (duplicated M16) so the full 32-row psum band is
                    # written; gap rows hold duplicate outputs, never stored.
                    stat = statp.tile([128, NLO, 32], bf16, name=f"st{c}", tag="st")
                    in0 = m32[:].unsqueeze(1).to_broadcast([128, NLO, 32])
                    in1 = e_t[:].unsqueeze(2).to_broadcast([128, NLO, 32])
                    nc.gpsimd.tensor_tensor(stat[:], in0, in1, op=mult)

                    # ---- pooling into band q of the super-chunk psum tile ----
                    band = pooled_sup[32 * q : 32 * q + 32, :]
                    for s in range(NLO):
                        nc.tensor.matmul(
                            band,
                            stat[:, s, :],
                            tb[:, s, :],
                            start=(s == 0),
                            stop=(s == NLO - 1),
                            tile_position=(0, 32 * q),
                        )

                    if q == SUP - 1:

                        def make_tail(ps=pooled_sup, ss=sums_sup, og=outg, cc=c):
                            def tail():
                                # one reciprocal covers all 4 bands in place
                                rcs = scp.tile([128, 1], f32, tag="rc")
                                nc.vector.reciprocal(rcs[:], ss[:])
                                # normalize each band in the PSUM->SBUF copy
                                for qq in range(SUP):
                                    nc.scalar.mul(
                                        og[32 * qq : 32 * qq + 32, :],
                                        ps[32 * qq : 32 * qq + 32, :],
                                        rcs[32 * qq : 32 * qq + 32, :],
                                    )
                                # issue from the Act hwdge queue: the SP queue
                                # must stay dedicated to the tm stream
                                ob = (cc - SUP + 1) * CH
                                for qq in range(SUP):
                                    nc.scalar.dma_start(
                                        out[ob + qq * CH : ob + (qq + 1) * CH, :],
                                        og[32 * qq : 32 * qq + CH, :],
                                    )

                            return tail

                        pending_tail[0] = make_tail()

            flush_tail()

    nc.compile()
    return nc


def _get_nc():
    global _NC
    if _NC is None:
        _NC = _build()
    return _NC


def _pack(near_emb, delta_xy, delta_cs):
    return np.concatenate(
        [
            np.asarray(near_emb, dtype=np.float32),
            np.asarray(delta_xy, dtype=np.float32),
            np.asarray(delta_cs, dtype=np.float32),
        ],
        axis=-1,
    )


def make_in_maps(near_emb, delta_xy, delta_cs, B_query, W_key):
    nearcat = _pack(near_emb, delta_xy, delta_cs)
    B_query = np.ascontiguousarray(np.asarray(B_query, dtype=np.float32))
    W_key = np.ascontiguousarray(np.asarray(W_key, dtype=np.float32))
    in_maps = []
    for c in range(CORES):
        s = slice(c * PB, (c + 1) * PB)
        in_maps.append(
            {
                "nearcat": nearcat[s],
                "bq": B_query[s],
                "wk": W_key,
            }
        )
    return in_maps


def kernel(near_emb, delta_xy, delta_cs, B_query, W_key, b_key=None, **_ignored):
    from concourse import bass_utils

    nc = _get_nc()
    in_maps = make_in_maps(near_emb, delta_xy, delta_cs, B_query, W_key)
    res = bass_utils.run_bass_kernel_spmd(nc, in_maps, core_ids=list(range(CORES)))
    return np.concatenate([res.results[c]["out"] for c in range(CORES)], axis=0)
